# revision 9
# baseline (speedup 1.0000x reference)
"""CapsNet forward Trainium2 Bass kernel (8-core data parallel).

Per core (B=32 of 256 samples):
  conv1 9x9 s1 (1->256) + ReLU           -> h   [256, 20, 20]
  primary caps conv 9x9 s2 (256->256)    -> p   [256, 6, 6]
  squash over 1152 per (b, i)            -> u   [b, 1152, 8]
  u_hat = einsum('bri,rico->brco', u, W) -> [b, 1152, 10, 16]
  3 dynamic-routing iterations           -> v   [b, 10, 16]

All matmuls bf16 with fp32 PSUM accumulation.  Primary-conv output channels
are column-reordered host-side so the conv psum lands directly in
partitions (rq, i); u then feeds a block-diagonal stationary
(K=(rq16,i8), M=(rq'16,b8)) whose diagonal is filled by one flat-address
strided DMA per (r-group, sample-group), zeros kept in 4 persistent memset
tiles.  u_hat lives as [p=(rq,b^), (g72, o16, c10)] bf16; r-reductions go
to PE via an S8 (p%8==j) matrix psum-accumulated over g; o-reductions are
a chunked DVE add-tree; broadcasts are stride-0 APs with c innermost so
DVE multiplies run in 2x bf16 mode.
"""

import numpy as np
import ml_dtypes

import concourse.bass as bass
import concourse.tile as tile
from concourse import bacc
from concourse import mybir
from concourse.ap import AP
from concourse.bass_utils import run_bass_kernel_spmd

BF = mybir.dt.bfloat16
F32 = mybir.dt.float32
AX = mybir.AxisListType
OP = mybir.AluOpType
ACTF = mybir.ActivationFunctionType

import os
STAGE = int(os.environ.get("KSTAGE", "99"))
NCORES = 8
B = 32            # samples per core
G = 4             # sample groups
BG = 8            # samples per group
NYX = 36          # primary caps spatial positions (6x6)
NG = 72           # r-groups of 16: g = (yx, h)
NC_ = 10          # digit caps count (c)
DO = 16           # digit caps dim (o)
CO = DO * NC_     # 160 cols (o, c), c innermost
GCOLS = NG * CO   # 11520 u_hat cols per group
CH = 8            # g's per routing chunk
NCH = NG // CH    # 6 chunks


def _bf(x):
    return np.asarray(x, dtype=ml_dtypes.bfloat16)


def build():
    nc = bacc.Bacc("TRN2", target_bir_lowering=False, debug=False)

    x1_d = nc.dram_tensor("x1", [81, 12800], BF, kind="ExternalInput").ap()
    w1_d = nc.dram_tensor("w1", [81, 256], BF, kind="ExternalInput").ap()
    b1_d = nc.dram_tensor("b1", [128, 2], F32, kind="ExternalInput").ap()
    # primary weights: [ich, ic128, oh2, k81, ocol128] (ocol = rq*8+i reorder)
    pw_d = nc.dram_tensor("pw", [2, 128, 2, 81, 128], BF, kind="ExternalInput").ap()
    pb_d = nc.dram_tensor("pb", [128, 2], F32, kind="ExternalInput").ap()
    wd_d = nc.dram_tensor("wd", [NG, 128, CO], BF, kind="ExternalInput").ap()
    s8_d = nc.dram_tensor("s8", [128, 8], BF, kind="ExternalInput").ap()
    dm_d = nc.dram_tensor("dmask", [128, 128], BF, kind="ExternalInput").ap()
    out_d = nc.dram_tensor("out", [B, NC_, DO], F32, kind="ExternalOutput").ap()

    with tile.TileContext(nc) as tc:
        _body(nc, tc, x1_d, w1_d, b1_d, pw_d, pb_d, wd_d, s8_d, dm_d, out_d)
    nc.compile()
    return nc


def _body(nc, tc, x1_d, w1_d, b1_d, pw_d, pb_d, wd_d, s8_d, dm_d, out_d):
    with (
        tc.tile_pool(name="const", bufs=1) as constp,
        tc.tile_pool(name="pwres", bufs=1) as pwresp,
        tc.tile_pool(name="big", bufs=2) as bigp,     # x1 + uhg share slots
        tc.tile_pool(name="h", bufs=1) as hp,
        tc.tile_pool(name="ub", bufs=4) as ubp,
        tc.tile_pool(name="wd", bufs=4) as wdp,
        tc.tile_pool(name="sm", bufs=2) as smp,
        tc.tile_pool(name="rt", bufs=2) as rtp,
        tc.tile_pool(name="psA", bufs=2, space="PSUM") as psA,   # conv1 [128,512]
        tc.tile_pool(name="psB", bufs=3, space="PSUM") as psB,   # prim [128,288]
        tc.tile_pool(name="psC", bufs=2, space="PSUM") as psC,   # u_hat [128,160]
        tc.tile_pool(name="psD", bufs=1, space="PSUM") as psD,   # small [8,x]
    ):
        # ---------------- constants ----------------
        w1 = constp.tile([81, 256], BF, tag="w1")
        nc.sync.dma_start(w1[:], w1_d[:])
        b1 = constp.tile([128, 2], F32, tag="b1")
        nc.sync.dma_start(b1[:], b1_d[:])
        pb = constp.tile([128, 2], F32, tag="pb")
        nc.sync.dma_start(pb[:], pb_d[:])
        s8 = constp.tile([128, 8], BF, tag="s8")
        nc.sync.dma_start(s8[:], s8_d[:])
        dmask = constp.tile([128, 128], BF, tag="dmask")
        nc.sync.dma_start(dmask[:], dm_d[:])
        pws = []
        for ich in range(2):
            pwt = pwresp.tile([128, 2 * 81 * 128], BF, tag=f"pw{ich}",
                              name=f"pw{ich}")
            nc.sync.dma_start(pwt[:], pw_d[ich].rearrange("p a b c -> p (a b c)"))
            pws.append(pwt)

        # ---------------- conv1 im2col loaded from host ----------------
        x1 = bigp.tile([81, 12800], BF, tag="big", name="x1")
        nc.sync.dma_start(x1[:], x1_d[:])

        # ---------------- conv1 (all samples) ----------------
        hs = []
        for oh in range(2):
            ht = hp.tile([128, 12800], BF, tag=f"h{oh}", name=f"h{oh}")
            hs.append(ht)
            for ci in range(25):
                pt = psA.tile([128, 512], F32, tag="c1", name="c1")
                nc.tensor.matmul(
                    pt[:], w1[:, oh * 128 : (oh + 1) * 128],
                    x1[:, ci * 512 : (ci + 1) * 512],
                    start=True, stop=True,
                )
                nc.scalar.activation(
                    ht[:, ci * 512 : (ci + 1) * 512], pt[:],
                    ACTF.Relu, bias=b1[:, oh : oh + 1],
                )

        if STAGE < 1:
            return
        for grp in range(G):
            # ============ primary caps conv ============
            pps = []
            for oh in range(2):
                pt = psB.tile([128, 288], F32, tag="pp", name="pp")
                pps.append(pt)
                first = True
                for k in range(81):
                    ky, kx = divmod(k, 9)
                    for ich in range(2):
                        lhs = pws[ich][:, (oh * 81 + k) * 128 : (oh * 81 + k + 1) * 128]
                        hr = hs[ich].rearrange("p (y x b) -> p y x b",
                                               y=20, x=20, b=B)
                        rhs = hr[:, ky : ky + 12 : 2, kx : kx + 12 : 2,
                                 grp * BG : (grp + 1) * BG]
                        nc.tensor.matmul(
                            pt[:], lhs, rhs,
                            start=first, stop=(k == 80 and ich == 1),
                        )
                        first = False

            if STAGE < 2:
                continue
            # ============ squash -> u ============
            us = []
            sqsum = smp.tile([128, 16], F32, tag="sqs", name="sqs")
            sq = smp.tile([128, 288], F32, tag="sq", name="sq", bufs=1)
            for oh in range(2):
                ut = smp.tile([128, NYX * BG], BF, tag=f"u{oh}", name=f"u{oh}")
                us.append(ut)
                nc.scalar.activation(ut[:], pps[oh][:], ACTF.Identity,
                                     bias=pb[:, oh : oh + 1])
                # sum over yx of (p + bias)^2
                nc.scalar.activation(sq[:], pps[oh][:], ACTF.Square,
                                     bias=pb[:, oh : oh + 1])
                nc.vector.tensor_reduce(
                    sqsum[:, oh * BG : (oh + 1) * BG],
                    sq.rearrange("p (yx b) -> p b yx", yx=NYX, b=BG),
                    axis=AX.X, op=OP.add)
            sqbf = smp.tile([128, 16], BF, tag="sqbf", name="sqbf")
            nc.vector.tensor_copy(sqbf[:], sqsum[:])
            nps = psD.tile([8, BG], F32, tag="smallps", name="nps")
            nc.tensor.matmul(nps[:], s8[:], sqbf[:, 0:8], start=True, stop=False)
            nc.tensor.matmul(nps[:], s8[:], sqbf[:, 8:16], start=False, stop=True)
            # scale[i,b] = sqrt(n)/(n+1)
            nsb = smp.tile([8, 3 * BG], F32, tag="nsb", name="nsb")
            nc.scalar.activation(nsb[:, 0:BG], nps[:], ACTF.Sqrt)
            nc.vector.tensor_scalar_add(nsb[:, BG:2 * BG], nps[:], 1.0)
            nc.vector.reciprocal(nsb[:, BG:2 * BG], nsb[:, BG:2 * BG])
            nc.vector.tensor_tensor(nsb[:, 2 * BG:3 * BG], nsb[:, 0:BG],
                                    nsb[:, BG:2 * BG], op=OP.mult)
            screp = smp.tile([128, BG], F32, tag="screp", name="screp")
            nc.vector.tensor_copy(screp[0:8, :], nsb[:, 2 * BG:3 * BG])
            for step in (8, 16, 32, 64):
                nc.sync.dma_start(
                    AP(screp.tensor, step * BG, [[BG, step], [1, BG]]),
                    AP(screp.tensor, 0, [[BG, step], [1, BG]]))
            for oh in range(2):
                nc.vector.tensor_tensor(
                    us[oh].rearrange("p (yx b) -> p yx b", yx=NYX, b=BG),
                    us[oh].rearrange("p (yx b) -> p yx b", yx=NYX, b=BG),
                    AP(screp.tensor, 0, [[BG, 128], [0, NYX], [1, BG]]),
                    op=OP.mult)

            if STAGE < 3:
                continue
            # ============ u_hat ============
            uhg = bigp.tile([128, GCOLS], BF, tag="big", name="uhg")
            for g in range(NG):
                yx, hh = divmod(g, 2)
                ub = ubp.tile([128, 128], BF, tag="ublk", name="ub")
                nc.vector.tensor_tensor(
                    ub[:],
                    AP(us[hh].tensor, yx * BG, [[NYX * BG, 128], [0, 16], [1, BG]]),
                    dmask[:], op=OP.mult)
                wdt = wdp.tile([128, CO], BF, tag="wd", name="wd")
                nc.sync.dma_start(wdt[:], wd_d[g])
                up = psC.tile([128, CO], F32, tag="uhp", name="uhp")
                nc.tensor.matmul(up[:], ub[:], wdt[:], start=True, stop=True)
                nc.vector.tensor_copy(uhg[:, g * CO : (g + 1) * CO], up[:])

            if STAGE < 4:
                continue
            # ============ routing ============
            _routing(nc, rtp, psD, s8, uhg, out_d, grp)


def _routing(nc, rtp, psp, s8, uhg, out_d, grp):
    """3 routing iterations for one group. uhg [p=(rq,b^8), (g72, o16, c10)]."""
    uht = uhg.tensor
    blog = rtp.tile([128, NG * NC_], BF, tag="blog", name="blog", bufs=1)
    ex = rtp.tile([128, NG * NC_], BF, tag="ex", name="ex", bufs=1)
    sden = rtp.tile([128, NC_], F32, tag="sden", name="sden")
    sdenb = rtp.tile([128, NC_], BF, tag="sdenb", name="sdenb")
    vrep = rtp.tile([128, CO], BF, tag="vrep", name="vrep")
    sm = rtp.tile([8, 640], F32, tag="sm", name="sm")
    smt = sm.tensor
    # sm: s[0:160] sq[160:320] n[320:330] d[330:340] sqr[340:350] sc[350:360]
    #     v[360:520] rec[520:530] vco[0:160 reuse at end]
    REC = 520

    for it in range(3):
        sps = psp.tile([8, CO], F32, tag="smallps", name="sps")
        if it == 0:
            for g in range(NG):
                nc.tensor.matmul(
                    sps[:], s8[:], uhg[:, g * CO : (g + 1) * CO],
                    start=(g == 0), stop=(g == NG - 1))
        else:
            for ci in range(NCH):
                c0 = ci * CH
                ab = rtp.tile([128, CH * CO], BF, tag="abuf", name="ab")
                nc.vector.tensor_tensor(
                    ab.rearrange("p (g o c) -> p g o c", g=CH, o=DO, c=NC_),
                    AP(uht, c0 * CO, [[GCOLS, 128], [CO, CH], [NC_, DO], [1, NC_]]),
                    AP(ex.tensor, c0 * NC_,
                       [[NG * NC_, 128], [NC_, CH], [0, DO], [1, NC_]]),
                    op=OP.mult)
                for gg in range(CH):
                    g = c0 + gg
                    nc.tensor.matmul(
                        sps[:], s8[:], ab[:, gg * CO : (gg + 1) * CO],
                        start=(g == 0), stop=(g == NG - 1))
        # s = s_raw * recip ; squash ; v
        s_ = sm[:, 0:CO]
        if it == 0:
            nc.vector.tensor_scalar_mul(s_, sps[:], 1.0 / 1152.0)
        else:
            nc.vector.tensor_tensor(
                s_, sps[:], AP(smt, REC, [[640, 8], [0, DO], [1, NC_]]),
                op=OP.mult)
        nc.vector.tensor_tensor(sm[:, 160:320], s_, s_, op=OP.mult)
        nc.vector.tensor_reduce(
            sm[:, 320:330], AP(smt, 160, [[640, 8], [1, NC_], [NC_, DO]]),
            axis=AX.X, op=OP.add)
        nc.scalar.activation(sm[:, 340:350], sm[:, 320:330], ACTF.Sqrt)
        nc.vector.tensor_scalar_add(sm[:, 330:340], sm[:, 320:330], 1.0)
        nc.vector.reciprocal(sm[:, 330:340], sm[:, 330:340])
        nc.vector.tensor_tensor(sm[:, 350:360], sm[:, 340:350],
                                sm[:, 330:340], op=OP.mult)
        nc.vector.tensor_tensor(
            sm[:, 360:520], s_, AP(smt, 350, [[640, 8], [0, DO], [1, NC_]]),
            op=OP.mult)

        if it == 2:
            nc.vector.tensor_copy(
                AP(smt, 0, [[640, 8], [DO, NC_], [1, DO]]),
                AP(smt, 360, [[640, 8], [1, NC_], [NC_, DO]]))
            nc.sync.dma_start(
                out_d[grp * BG : (grp + 1) * BG],
                AP(smt, 0, [[640, 8], [DO, NC_], [1, DO]]))
            return

        # vrep: v (o,c) bf16 replicated over rq
        nc.vector.tensor_copy(vrep[0:8, :], sm[:, 360:520])
        for step in (8, 16, 32, 64):
            nc.sync.dma_start(
                AP(vrep.tensor, step * CO, [[CO, step], [1, CO]]),
                AP(vrep.tensor, 0, [[CO, step], [1, CO]]))
        # delta_b[p, (g, c)] = sum_o u_hat * vrep  (chunked mult + o-add-tree)
        for ci in range(NCH):
            c0 = ci * CH
            ab = rtp.tile([128, CH * CO], BF, tag="abuf", name="ab2")
            nc.vector.tensor_tensor(
                ab.rearrange("p (g o c) -> p g o c", g=CH, o=DO, c=NC_),
                AP(uht, c0 * CO, [[GCOLS, 128], [CO, CH], [NC_, DO], [1, NC_]]),
                AP(vrep.tensor, 0, [[CO, 128], [0, CH], [NC_, DO], [1, NC_]]),
                op=OP.mult)
            t1 = rtp.tile([128, CH * 8 * NC_], BF, tag="tr1", name="t1", bufs=1)
            nc.vector.tensor_tensor(
                t1[:],
                AP(ab.tensor, 0, [[CH * CO, 128], [CO, CH], [NC_, 8], [1, NC_]]),
                AP(ab.tensor, 8 * NC_,
                   [[CH * CO, 128], [CO, CH], [NC_, 8], [1, NC_]]),
                op=OP.add)
            t2 = rtp.tile([128, CH * 4 * NC_], BF, tag="tr2", name="t2", bufs=1)
            nc.vector.tensor_tensor(
                t2[:],
                AP(t1.tensor, 0, [[CH * 8 * NC_, 128], [8 * NC_, CH], [NC_, 4], [1, NC_]]),
                AP(t1.tensor, 4 * NC_,
                   [[CH * 8 * NC_, 128], [8 * NC_, CH], [NC_, 4], [1, NC_]]),
                op=OP.add)
            t3 = rtp.tile([128, CH * 2 * NC_], BF, tag="tr3", name="t3", bufs=1)
            nc.vector.tensor_tensor(
                t3[:],
                AP(t2.tensor, 0, [[CH * 4 * NC_, 128], [4 * NC_, CH], [NC_, 2], [1, NC_]]),
                AP(t2.tensor, 2 * NC_,
                   [[CH * 4 * NC_, 128], [4 * NC_, CH], [NC_, 2], [1, NC_]]),
                op=OP.add)
            t3lo = AP(t3.tensor, 0, [[CH * 2 * NC_, 128], [2 * NC_, CH], [1, NC_]])
            t3hi = AP(t3.tensor, NC_, [[CH * 2 * NC_, 128], [2 * NC_, CH], [1, NC_]])
            bsl = blog[:, c0 * NC_ : (c0 + CH) * NC_]
            if it == 0:
                nc.vector.tensor_tensor(bsl, t3lo, t3hi, op=OP.add)
            else:
                t4 = rtp.tile([128, CH * NC_], BF, tag="tr4", name="t4", bufs=1)
                nc.vector.tensor_tensor(t4[:], t3lo, t3hi, op=OP.add)
                nc.vector.tensor_tensor(bsl, bsl, t4[:], op=OP.add)
        # softmax pieces for next iteration
        nc.scalar.activation(ex[:], blog[:], ACTF.Exp)
        nc.vector.tensor_reduce(
            sden[:], AP(ex.tensor, 0, [[NG * NC_, 128], [1, NC_], [NC_, NG]]),
            axis=AX.X, op=OP.add)
        nc.vector.tensor_copy(sdenb[:], sden[:])
        dps = psp.tile([8, NC_], F32, tag="smallps", name="dps")
        nc.tensor.matmul(dps[:], s8[:], sdenb[:], start=True, stop=True)
        nc.vector.reciprocal(sm[:, REC:REC + NC_], dps[:])


# ============================================================
# host side
# ============================================================
_CACHE = {}


def _prep(inputs):
    x = np.asarray(inputs["x"], np.float32)
    conv1_w = np.asarray(inputs["conv1_w"], np.float32)
    conv1_b = np.asarray(inputs["conv1_b"], np.float32)
    prim_w = np.asarray(inputs["prim_w"], np.float32)
    prim_b = np.asarray(inputs["prim_b"], np.float32)
    W_digit = np.asarray(inputs["W_digit"], np.float32)

    w1 = _bf(np.ascontiguousarray(conv1_w.reshape(256, 81).T))
    b1 = np.ascontiguousarray(conv1_b.reshape(2, 128).T)

    j = np.arange(128)
    rq, i = j // 8, j % 8
    pw = prim_w.reshape(256, 256, 81)
    pwt = np.zeros((2, 128, 2, 81, 128), np.float32)  # [ich, ic, oh, k, ocol]
    pb2 = np.zeros(256, np.float32)
    pbv = prim_b.reshape(256)
    for oh in range(2):
        sel = i * 32 + oh * 16 + rq
        pb2[oh * 128 : (oh + 1) * 128] = pbv[sel]
        w_oh = pw[sel]                        # [128ocol, 256ic, 81k]
        for ich in range(2):
            pwt[ich, :, oh] = w_oh[:, ich * 128 : (ich + 1) * 128, :].transpose(1, 2, 0)
    pwt = _bf(pwt)

    wd = W_digit.reshape(2, 16, 36, 8, NC_, DO)       # [h, rq, yx, i, c, o]
    wd = wd.transpose(2, 0, 1, 3, 5, 4)               # [yx, h, rq, i, o, c]
    wd = _bf(np.ascontiguousarray(wd.reshape(NG, 128, CO)))

    s8m = np.zeros((128, 8), np.float32)
    s8m[np.arange(128), np.arange(128) % 8] = 1.0
    s8m = _bf(s8m)
    dm = np.zeros((128, 128), np.float32)
    for p in range(128):
        rq = p // 8
        dm[p, rq * 8 : rq * 8 + 8] = 1.0
    dm = _bf(dm)

    in_maps = []
    for core in range(NCORES):
        xc = x[core * B : (core + 1) * B, 0]              # [32, 28, 28]
        x1c = np.empty((81, 20, 20, B), np.float32)       # [k, y, x, b]
        for ky in range(9):
            for kx in range(9):
                x1c[ky * 9 + kx] = xc[:, ky:ky + 20, kx:kx + 20].transpose(1, 2, 0)
        in_maps.append({
            "x1": _bf(x1c.reshape(81, 12800)), "w1": w1, "b1": b1,
            "pw": pwt, "pb": np.ascontiguousarray(pb2.reshape(2, 128).T),
            "wd": wd, "s8": s8m, "dmask": dm,
        })
    return in_maps


def kernel(**inputs):
    if "nc" not in _CACHE:
        _CACHE["nc"] = build()
    nc = _CACHE["nc"]
    in_maps = _prep(inputs)
    res = run_bass_kernel_spmd(nc, in_maps, list(range(NCORES)))
    out = np.concatenate([res.results[i]["out"] for i in range(NCORES)], axis=0)
    return out.astype(np.float32)


if __name__ == "__main__":
    build()
    print("build OK")


# revision 13
# speedup vs baseline: 20.0508x; 20.0508x over previous
"""CapsNet forward Trainium2 Bass kernel (8-core data parallel).

Per core (B=32 of 256 samples):
  conv1 9x9 s1 (1->256) + ReLU           -> h   [256, 20, 20]
  primary caps conv 9x9 s2 (256->256)    -> p   [256, 6, 6]
  squash over 1152 per (b, i)            -> u   [b, 1152, 8]
  u_hat = einsum('bri,rico->brco', u, W) -> [b, 1152, 10, 16]
  3 dynamic-routing iterations           -> v   [b, 10, 16]

All matmuls bf16 with fp32 PSUM accumulation.  Primary-conv output channels
are column-reordered host-side so the conv psum lands directly in
partitions (rq, i); u then feeds a block-diagonal stationary
(K=(rq16,i8), M=(rq'16,b8)) whose diagonal is filled by one flat-address
strided DMA per (r-group, sample-group), zeros kept in 4 persistent memset
tiles.  u_hat lives as [p=(rq,b^), (g72, o16, c10)] bf16; r-reductions go
to PE via an S8 (p%8==j) matrix psum-accumulated over g; o-reductions are
a chunked DVE add-tree; broadcasts are stride-0 APs with c innermost so
DVE multiplies run in 2x bf16 mode.
"""

import numpy as np
import ml_dtypes

import concourse.bass as bass
import concourse.tile as tile
from concourse import bacc
from concourse import mybir
from concourse.ap import AP
from concourse.bass_utils import run_bass_kernel_spmd

BF = mybir.dt.bfloat16
F32 = mybir.dt.float32
AX = mybir.AxisListType
OP = mybir.AluOpType
ACTF = mybir.ActivationFunctionType

import os
STAGE = int(os.environ.get("KSTAGE", "99"))
NCORES = 8
B = 32            # samples per core
G = 4             # sample groups
BG = 8            # samples per group
NYX = 36          # primary caps spatial positions (6x6)
NG = 72           # r-groups of 16: g = (yx, h)
NC_ = 10          # digit caps count (c)
DO = 16           # digit caps dim (o)
CO = DO * NC_     # 160 cols (o, c), c innermost
GCOLS = NG * CO   # 11520 u_hat cols per group
CH = 12           # g's per routing chunk
NCH = NG // CH    # 6 chunks


def _bf(x):
    return np.asarray(x, dtype=ml_dtypes.bfloat16)


def build():
    nc = bacc.Bacc("TRN2", target_bir_lowering=False, debug=False)

    x1_d = nc.dram_tensor("x1", [81, 12800], BF, kind="ExternalInput").ap()
    w1_d = nc.dram_tensor("w1", [81, 256], BF, kind="ExternalInput").ap()
    b1_d = nc.dram_tensor("b1", [128, 2], F32, kind="ExternalInput").ap()
    # primary weights: [ich, ic128, oh2, k81, ocol128] (ocol = rq*8+i reorder)
    pw_d = nc.dram_tensor("pw", [2, 128, 2, 81, 128], BF, kind="ExternalInput").ap()
    pb_d = nc.dram_tensor("pb", [128, 2], F32, kind="ExternalInput").ap()
    wd_d = nc.dram_tensor("wd", [NG, 128, CO], BF, kind="ExternalInput").ap()
    s8_d = nc.dram_tensor("s8", [128, 8], BF, kind="ExternalInput").ap()
    dm_d = nc.dram_tensor("dmask", [128, 128], BF, kind="ExternalInput").ap()
    out_d = nc.dram_tensor("out", [B, NC_, DO], F32, kind="ExternalOutput").ap()

    with tile.TileContext(nc) as tc:
        _body(nc, tc, x1_d, w1_d, b1_d, pw_d, pb_d, wd_d, s8_d, dm_d, out_d)
    nc.compile()
    return nc


def _body(nc, tc, x1_d, w1_d, b1_d, pw_d, pb_d, wd_d, s8_d, dm_d, out_d):
    with (
        tc.tile_pool(name="const", bufs=1) as constp,
        tc.tile_pool(name="pwres", bufs=1) as pwresp,
        tc.tile_pool(name="big", bufs=2) as bigp,     # x1 + uhg share slots
        tc.tile_pool(name="h", bufs=1) as hp,
        tc.tile_pool(name="ub", bufs=2) as ubp,
        tc.tile_pool(name="wd", bufs=2) as wdp,
        tc.tile_pool(name="sm", bufs=2) as smp,
        tc.tile_pool(name="rt", bufs=2) as rtp,
        tc.tile_pool(name="psA", bufs=2, space="PSUM") as psA,   # conv1 [128,512]
        tc.tile_pool(name="psB", bufs=2, space="PSUM") as psB,   # prim [128,288]
        tc.tile_pool(name="psC", bufs=2, space="PSUM") as psC,   # u_hat [128,160]
        tc.tile_pool(name="psD", bufs=1, space="PSUM") as psD,   # small [8,x]
    ):
        # ---------------- constants ----------------
        w1 = constp.tile([81, 256], BF, tag="w1")
        nc.sync.dma_start(w1[:], w1_d[:])
        b1 = constp.tile([128, 2], F32, tag="b1")
        nc.sync.dma_start(b1[:], b1_d[:])
        pb = constp.tile([128, 2], F32, tag="pb")
        nc.sync.dma_start(pb[:], pb_d[:])
        s8 = constp.tile([128, 8], BF, tag="s8")
        nc.sync.dma_start(s8[:], s8_d[:])
        dm4 = constp.tile([128, 512], BF, tag="dm4")
        nc.sync.dma_start(dm4[:].rearrange("p (r m) -> p r m", r=4, m=128),
                          dm_d[:].unsqueeze(1).broadcast_to([128, 4, 128]))
        pws = []
        for ich in range(2):
            pwt = pwresp.tile([128, 2 * 81 * 128], BF, tag=f"pw{ich}",
                              name=f"pw{ich}")
            nc.sync.dma_start(pwt[:], pw_d[ich].rearrange("p a b c -> p (a b c)"))
            pws.append(pwt)

        # ---------------- conv1 im2col loaded from host ----------------
        x1 = bigp.tile([81, 12800], BF, tag="big", name="x1")
        nc.sync.dma_start(x1[:], x1_d[:])

        # ---------------- conv1 (all samples) ----------------
        hs = []
        for oh in range(2):
            ht = hp.tile([128, 12800], BF, tag=f"h{oh}", name=f"h{oh}")
            hs.append(ht)
            for ci in range(25):
                pt = psA.tile([128, 512], F32, tag="c1", name="c1")
                nc.tensor.matmul(
                    pt[:], w1[:, oh * 128 : (oh + 1) * 128],
                    x1[:, ci * 512 : (ci + 1) * 512],
                    start=True, stop=True,
                )
                nc.scalar.activation(
                    ht[:, ci * 512 : (ci + 1) * 512], pt[:],
                    ACTF.Relu, bias=b1[:, oh : oh + 1],
                )

        if STAGE < 1:
            return
        for grp in range(G):
            # ============ primary caps conv ============
            pps = []
            for oh in range(2):
                pt = psB.tile([128, 288], F32, tag="pp", name="pp")
                pps.append(pt)
                first = True
                for k in range(81):
                    ky, kx = divmod(k, 9)
                    for ich in range(2):
                        lhs = pws[ich][:, (oh * 81 + k) * 128 : (oh * 81 + k + 1) * 128]
                        hr = hs[ich].rearrange("p (y x b) -> p y x b",
                                               y=20, x=20, b=B)
                        rhs = hr[:, ky : ky + 12 : 2, kx : kx + 12 : 2,
                                 grp * BG : (grp + 1) * BG]
                        nc.tensor.matmul(
                            pt[:], lhs, rhs,
                            start=first, stop=(k == 80 and ich == 1),
                        )
                        first = False

            if STAGE < 2:
                continue
            # ============ squash -> u ============
            us = []
            sqsum = smp.tile([128, 16], F32, tag="sqs", name="sqs")
            sq = smp.tile([128, 288], F32, tag="sq", name="sq", bufs=1)
            for oh in range(2):
                ut = smp.tile([128, NYX * BG], BF, tag=f"u{oh}", name=f"u{oh}")
                us.append(ut)
                nc.scalar.activation(ut[:], pps[oh][:], ACTF.Identity,
                                     bias=pb[:, oh : oh + 1])
                # sum over yx of (p + bias)^2
                nc.scalar.activation(sq[:], pps[oh][:], ACTF.Square,
                                     bias=pb[:, oh : oh + 1])
                nc.vector.tensor_reduce(
                    sqsum[:, oh * BG : (oh + 1) * BG],
                    sq.rearrange("p (yx b) -> p b yx", yx=NYX, b=BG),
                    axis=AX.X, op=OP.add)
            sqbf = smp.tile([128, 16], BF, tag="sqbf", name="sqbf")
            nc.vector.tensor_copy(sqbf[:], sqsum[:])
            nps = psD.tile([8, BG], F32, tag="smallps", name="nps")
            nc.tensor.matmul(nps[:], s8[:], sqbf[:, 0:8], start=True, stop=False)
            nc.tensor.matmul(nps[:], s8[:], sqbf[:, 8:16], start=False, stop=True)
            # scale[i,b] = sqrt(n)/(n+1)
            nsb = smp.tile([8, 3 * BG], F32, tag="nsb", name="nsb")
            nc.scalar.activation(nsb[:, 0:BG], nps[:], ACTF.Sqrt)
            nc.vector.tensor_scalar_add(nsb[:, BG:2 * BG], nps[:], 1.0)
            nc.vector.reciprocal(nsb[:, BG:2 * BG], nsb[:, BG:2 * BG])
            nc.vector.tensor_tensor(nsb[:, 2 * BG:3 * BG], nsb[:, 0:BG],
                                    nsb[:, BG:2 * BG], op=OP.mult)
            screp = smp.tile([128, BG], F32, tag="screp", name="screp")
            nc.vector.tensor_copy(screp[0:8, :], nsb[:, 2 * BG:3 * BG])
            for step in (8, 16, 32, 64):
                nc.sync.dma_start(
                    AP(screp.tensor, step * BG, [[BG, step], [1, BG]]),
                    AP(screp.tensor, 0, [[BG, step], [1, BG]]))
            for oh in range(2):
                nc.vector.tensor_tensor(
                    us[oh].rearrange("p (yx b) -> p yx b", yx=NYX, b=BG),
                    us[oh].rearrange("p (yx b) -> p yx b", yx=NYX, b=BG),
                    AP(screp.tensor, 0, [[BG, 128], [0, NYX], [1, BG]]),
                    op=OP.mult)

            if STAGE < 3:
                continue
            # ============ u_hat ============
            uhg = bigp.tile([128, GCOLS], BF, tag="big", name="uhg")
            # g order: g = hh*36 + yx  (quads share hh for 4-wide mask-mult)
            for q in range(NG // 4):
                hh = (4 * q) // 36
                yx0 = (4 * q) % 36
                ub = ubp.tile([128, 512], BF, tag="ublk", name="ub")
                nc.vector.tensor_tensor(
                    ub[:].rearrange("p (blk m) -> p blk m", blk=4, m=128),
                    AP(us[hh].tensor, yx0 * BG,
                       [[NYX * BG, 128], [BG, 4], [0, 16], [1, BG]]),
                    dm4[:].rearrange("p (blk m) -> p blk m", blk=4, m=128),
                    op=OP.mult)
                wdt = wdp.tile([128, 4 * CO], BF, tag="wd", name="wd")
                nc.sync.dma_start(
                    wdt[:],
                    AP(wd_d.tensor, 4 * q * 128 * CO,
                       [[CO, 128], [128 * CO, 4], [1, CO]]))
                up = psC.tile([128, 3 * CO], F32, tag="uhp", name="uhp")
                up2 = psC.tile([128, CO], F32, tag="uhp2", name="uhp2", bufs=1)
                for j in range(4):
                    dst = up[:, j * CO : (j + 1) * CO] if j < 3 else up2[:]
                    nc.tensor.matmul(
                        dst, ub[:, j * 128 : (j + 1) * 128],
                        wdt[:, j * CO : (j + 1) * CO],
                        start=(j == 0 or j == 3), stop=(j == 2 or j == 3),
                        skip_group_check=True)
                nc.vector.tensor_copy(
                    uhg[:, 4 * q * CO : (4 * q + 3) * CO], up[:])
                nc.scalar.copy(
                    uhg[:, (4 * q + 3) * CO : (4 * q + 4) * CO], up2[:])

            if STAGE < 4:
                continue
            # ============ routing ============
            _routing(nc, rtp, psD, s8, uhg, out_d, grp)


def _routing(nc, rtp, psp, s8, uhg, out_d, grp):
    """3 routing iterations for one group. uhg [p=(rq,b^8), (g72, o16, c10)]."""
    uht = uhg.tensor
    blog = rtp.tile([128, NG * NC_], BF, tag="blog", name="blog", bufs=1)
    ex = rtp.tile([128, NG * NC_], BF, tag="ex", name="ex", bufs=1)
    sden = rtp.tile([128, NC_], F32, tag="sden", name="sden")
    sdenb = rtp.tile([128, NC_], BF, tag="sdenb", name="sdenb")
    vrep = rtp.tile([128, CO], BF, tag="vrep", name="vrep")
    sm = rtp.tile([8, 640], F32, tag="sm", name="sm", bufs=1)
    smt = sm.tensor
    # sm: s[0:160] sq[160:320] n[320:330] d[330:340] sqr[340:350] sc[350:360]
    #     v[360:520] rec[520:530] vco[0:160 reuse at end]
    REC = 520

    for it in range(3):
        sps = psp.tile([8, CO], F32, tag="smallps", name="sps")
        if it == 0:
            for g in range(NG):
                nc.tensor.matmul(
                    sps[:], s8[:], uhg[:, g * CO : (g + 1) * CO],
                    start=(g == 0), stop=(g == NG - 1))
        else:
            for ci in range(NCH):
                c0 = ci * CH
                ab = rtp.tile([128, CH * CO], BF, tag="abuf", name="ab")
                nc.vector.tensor_tensor(
                    ab.rearrange("p (g o c) -> p g o c", g=CH, o=DO, c=NC_),
                    AP(uht, c0 * CO, [[GCOLS, 128], [CO, CH], [NC_, DO], [1, NC_]]),
                    AP(ex.tensor, c0 * NC_,
                       [[NG * NC_, 128], [NC_, CH], [0, DO], [1, NC_]]),
                    op=OP.mult)
                for gg in range(CH):
                    g = c0 + gg
                    nc.tensor.matmul(
                        sps[:], s8[:], ab[:, gg * CO : (gg + 1) * CO],
                        start=(g == 0), stop=(g == NG - 1))
        # s = s_raw * recip ; squash ; v
        s_ = sm[:, 0:CO]
        if it == 0:
            nc.vector.tensor_scalar_mul(s_, sps[:], 1.0 / 1152.0)
        else:
            nc.vector.tensor_tensor(
                s_, sps[:], AP(smt, REC, [[640, 8], [0, DO], [1, NC_]]),
                op=OP.mult)
        nc.vector.tensor_tensor(sm[:, 160:320], s_, s_, op=OP.mult)
        nc.vector.tensor_reduce(
            sm[:, 320:330], AP(smt, 160, [[640, 8], [1, NC_], [NC_, DO]]),
            axis=AX.X, op=OP.add)
        nc.scalar.activation(sm[:, 340:350], sm[:, 320:330], ACTF.Sqrt)
        nc.vector.tensor_scalar_add(sm[:, 330:340], sm[:, 320:330], 1.0)
        nc.vector.reciprocal(sm[:, 330:340], sm[:, 330:340])
        nc.vector.tensor_tensor(sm[:, 350:360], sm[:, 340:350],
                                sm[:, 330:340], op=OP.mult)
        nc.vector.tensor_tensor(
            sm[:, 360:520], s_, AP(smt, 350, [[640, 8], [0, DO], [1, NC_]]),
            op=OP.mult)

        if it == 2:
            nc.vector.tensor_copy(
                AP(smt, 0, [[640, 8], [DO, NC_], [1, DO]]),
                AP(smt, 360, [[640, 8], [1, NC_], [NC_, DO]]))
            nc.sync.dma_start(
                out_d[grp * BG : (grp + 1) * BG],
                AP(smt, 0, [[640, 8], [DO, NC_], [1, DO]]))
            return

        # vrep: v (o,c) bf16 replicated over rq
        nc.vector.tensor_copy(vrep[0:8, :], sm[:, 360:520])
        for step in (8, 16, 32, 64):
            nc.sync.dma_start(
                AP(vrep.tensor, step * CO, [[CO, step], [1, CO]]),
                AP(vrep.tensor, 0, [[CO, step], [1, CO]]))
        # delta_b[p, (g, c)] = sum_o u_hat * vrep  (chunked mult + o-add-tree)
        for ci in range(NCH):
            c0 = ci * CH
            ab = rtp.tile([128, CH * CO], BF, tag="abuf", name="ab2")
            nc.vector.tensor_tensor(
                ab.rearrange("p (g o c) -> p g o c", g=CH, o=DO, c=NC_),
                AP(uht, c0 * CO, [[GCOLS, 128], [CO, CH], [NC_, DO], [1, NC_]]),
                AP(vrep.tensor, 0, [[CO, 128], [0, CH], [NC_, DO], [1, NC_]]),
                op=OP.mult)
            t1 = rtp.tile([128, CH * 8 * NC_], BF, tag="tr1", name="t1", bufs=1)
            nc.vector.tensor_tensor(
                t1[:],
                AP(ab.tensor, 0, [[CH * CO, 128], [CO, CH], [NC_, 8], [1, NC_]]),
                AP(ab.tensor, 8 * NC_,
                   [[CH * CO, 128], [CO, CH], [NC_, 8], [1, NC_]]),
                op=OP.add)
            t2 = rtp.tile([128, CH * 4 * NC_], BF, tag="tr2", name="t2", bufs=1)
            nc.vector.tensor_tensor(
                t2[:],
                AP(t1.tensor, 0, [[CH * 8 * NC_, 128], [8 * NC_, CH], [NC_, 4], [1, NC_]]),
                AP(t1.tensor, 4 * NC_,
                   [[CH * 8 * NC_, 128], [8 * NC_, CH], [NC_, 4], [1, NC_]]),
                op=OP.add)
            t3 = rtp.tile([128, CH * 2 * NC_], BF, tag="tr3", name="t3", bufs=1)
            nc.vector.tensor_tensor(
                t3[:],
                AP(t2.tensor, 0, [[CH * 4 * NC_, 128], [4 * NC_, CH], [NC_, 2], [1, NC_]]),
                AP(t2.tensor, 2 * NC_,
                   [[CH * 4 * NC_, 128], [4 * NC_, CH], [NC_, 2], [1, NC_]]),
                op=OP.add)
            t3lo = AP(t3.tensor, 0, [[CH * 2 * NC_, 128], [2 * NC_, CH], [1, NC_]])
            t3hi = AP(t3.tensor, NC_, [[CH * 2 * NC_, 128], [2 * NC_, CH], [1, NC_]])
            bsl = blog[:, c0 * NC_ : (c0 + CH) * NC_]
            if it == 0:
                nc.vector.tensor_tensor(bsl, t3lo, t3hi, op=OP.add)
            else:
                t4 = rtp.tile([128, CH * NC_], BF, tag="tr2", name="t4", bufs=1)
                nc.vector.tensor_tensor(t4[:], t3lo, t3hi, op=OP.add)
                nc.vector.tensor_tensor(bsl, bsl, t4[:], op=OP.add)
        # softmax pieces for next iteration
        nc.scalar.activation(ex[:], blog[:], ACTF.Exp)
        nc.vector.tensor_reduce(
            sden[:], AP(ex.tensor, 0, [[NG * NC_, 128], [1, NC_], [NC_, NG]]),
            axis=AX.X, op=OP.add)
        nc.vector.tensor_copy(sdenb[:], sden[:])
        dps = psp.tile([8, NC_], F32, tag="smallps", name="dps")
        nc.tensor.matmul(dps[:], s8[:], sdenb[:], start=True, stop=True)
        nc.vector.reciprocal(sm[:, REC:REC + NC_], dps[:])


# ============================================================
# host side
# ============================================================
_CACHE = {}


def _prep(inputs):
    x = np.asarray(inputs["x"], np.float32)
    conv1_w = np.asarray(inputs["conv1_w"], np.float32)
    conv1_b = np.asarray(inputs["conv1_b"], np.float32)
    prim_w = np.asarray(inputs["prim_w"], np.float32)
    prim_b = np.asarray(inputs["prim_b"], np.float32)
    W_digit = np.asarray(inputs["W_digit"], np.float32)

    w1 = _bf(np.ascontiguousarray(conv1_w.reshape(256, 81).T))
    b1 = np.ascontiguousarray(conv1_b.reshape(2, 128).T)

    j = np.arange(128)
    rq, i = j // 8, j % 8
    pw = prim_w.reshape(256, 256, 81)
    pwt = np.zeros((2, 128, 2, 81, 128), np.float32)  # [ich, ic, oh, k, ocol]
    pb2 = np.zeros(256, np.float32)
    pbv = prim_b.reshape(256)
    for oh in range(2):
        sel = i * 32 + oh * 16 + rq
        pb2[oh * 128 : (oh + 1) * 128] = pbv[sel]
        w_oh = pw[sel]                        # [128ocol, 256ic, 81k]
        for ich in range(2):
            pwt[ich, :, oh] = w_oh[:, ich * 128 : (ich + 1) * 128, :].transpose(1, 2, 0)
    pwt = _bf(pwt)

    wd = W_digit.reshape(2, 16, 36, 8, NC_, DO)       # [h, rq, yx, i, c, o]
    wd = wd.transpose(0, 2, 1, 3, 5, 4)               # [h, yx, rq, i, o, c]
    wd = _bf(np.ascontiguousarray(wd.reshape(NG, 128, CO)))

    s8m = np.zeros((128, 8), np.float32)
    s8m[np.arange(128), np.arange(128) % 8] = 1.0
    s8m = _bf(s8m)
    dm = np.zeros((128, 128), np.float32)
    for p in range(128):
        rq = p // 8
        dm[p, rq * 8 : rq * 8 + 8] = 1.0
    dm = _bf(dm)

    in_maps = []
    for core in range(NCORES):
        xc = x[core * B : (core + 1) * B, 0]              # [32, 28, 28]
        x1c = np.empty((81, 20, 20, B), np.float32)       # [k, y, x, b]
        for ky in range(9):
            for kx in range(9):
                x1c[ky * 9 + kx] = xc[:, ky:ky + 20, kx:kx + 20].transpose(1, 2, 0)
        in_maps.append({
            "x1": _bf(x1c.reshape(81, 12800)), "w1": w1, "b1": b1,
            "pw": pwt, "pb": np.ascontiguousarray(pb2.reshape(2, 128).T),
            "wd": wd, "s8": s8m, "dmask": dm,
        })
    return in_maps


def kernel(**inputs):
    if "nc" not in _CACHE:
        _CACHE["nc"] = build()
    nc = _CACHE["nc"]
    in_maps = _prep(inputs)
    res = run_bass_kernel_spmd(nc, in_maps, list(range(NCORES)))
    out = np.concatenate([res.results[i]["out"] for i in range(NCORES)], axis=0)
    return out.astype(np.float32)


if __name__ == "__main__":
    build()
    print("build OK")
